# revision 1
# baseline (speedup 1.0000x reference)
"""MoE layer (16 experts, top-2) + shared SwiGLU MLP on 8 trn2 NeuronCores.

Sharding:
  - Expert-parallel: core c owns experts {2c, 2c+1}. Host computes the router
    (0.2% of FLOPs), gathers each expert's tokens ("all-to-all" done during
    input sharding), device runs the two expert FFNs on the gathered tokens.
  - Shared SwiGLU MLP: token-parallel, core c handles tokens [c*512,(c+1)*512).
  - Host applies the top-2 softmax combine weights and scatter-adds expert
    outputs back, then adds the shared-expert output.

All device matmuls are fp32 (PSUM fp32 accumulate). Weights are pre-blocked on
the host into contiguous [128,128] tile grids so every DMA is a contiguous
64KB+ transfer and every matmul is lhsT.T @ rhs with no on-device transposes.
"""

import os
import numpy as np

import concourse.bacc as bacc
import concourse.mybir as mybir
import concourse.tile as tile
from concourse import bass_utils

AF = mybir.ActivationFunctionType
FP32 = mybir.dt.float32

B, L, D, H, E, S = 2, 2048, 1024, 512, 16, 2048
T = B * L
TOP_K = 2
NCORES = 8
EPC = E // NCORES  # experts per core
TS = T // NCORES   # shared-expert tokens per core

KD = D // 128   # 8  contraction tiles over D
KS = S // 128   # 16 contraction tiles over S
KH = H // 128   # 4  contraction tiles over H

TRACE = False      # set True (or BASS_TRACE=1) to collect an NTFF profile
LAST = None        # BassKernelResults of the most recent run (for test.py)

_PROG_CACHE = {}


def _chunks(total, step=512):
    out = []
    off = 0
    while off < total:
        w = min(step, total - off)
        out.append((off, w))
        off += w
    return out


def _block(a, km, mm):
    """[K, M] k-major matrix -> [M/mm, K/km, km, mm] contiguous tile grid."""
    K, M = a.shape
    return np.ascontiguousarray(
        a.reshape(K // km, km, M // mm, mm).transpose(2, 0, 1, 3)
    )


def build_program(C):
    nc = bacc.Bacc(
        "TRN2", target_bir_lowering=False, debug=False, enable_asserts=False
    )

    xs = nc.dram_tensor("xs", [KD, 128, TS], FP32, kind="ExternalInput").ap()
    sfc1b = nc.dram_tensor("sfc1b", [KS, KD, 128, 128], FP32, kind="ExternalInput").ap()
    sfc2b = nc.dram_tensor("sfc2b", [KS, KD, 128, 128], FP32, kind="ExternalInput").ap()
    sfc3b = nc.dram_tensor("sfc3b", [KD, KS, 128, 128], FP32, kind="ExternalInput").ap()
    xg = nc.dram_tensor("xg", [EPC, KD, 128, C], FP32, kind="ExternalInput").ap()
    w1b = nc.dram_tensor("w1b", [EPC, KH, KD, 128, 128], FP32, kind="ExternalInput").ap()
    w2b = nc.dram_tensor("w2b", [EPC, KD, KH, 128, 128], FP32, kind="ExternalInput").ap()
    shout = nc.dram_tensor("shout", [KD, 128, TS], FP32, kind="ExternalOutput").ap()
    yout = nc.dram_tensor("yout", [EPC, KD, 128, C], FP32, kind="ExternalOutput").ap()

    with tile.TileContext(nc) as tc:
        with (
            tc.tile_pool(name="xsp", bufs=1) as xsp,
            tc.tile_pool(name="gp", bufs=1) as gp,
            tc.tile_pool(name="wap", bufs=2) as wap,
            tc.tile_pool(name="wbp", bufs=2) as wbp,
            tc.tile_pool(name="w3p", bufs=2) as w3p,
            tc.tile_pool(name="w1p", bufs=2) as w1p,
            tc.tile_pool(name="w2p", bufs=2) as w2p,
            tc.tile_pool(name="xep", bufs=2) as xep,
            tc.tile_pool(name="hp", bufs=2) as hp,
            tc.tile_pool(name="sap", bufs=3) as sap,
            tc.tile_pool(name="obp", bufs=4) as obp,
            tc.tile_pool(name="ps", bufs=6, space="PSUM") as ps,
        ):
            # resident: x shard (d-major) for the shared expert
            xs_t = xsp.tile([128, KD * TS], FP32)
            for j in range(KD):
                nc.sync.dma_start(out=xs_t[:, j * TS:(j + 1) * TS], in_=xs[j])

            # g[s, t] = silu(x@sfc1.T) * (x@sfc2.T), laid out s-tile-major
            g_t = gp.tile([128, KS * TS], FP32)
            for st in range(KS):
                wa = wap.tile([128, KD * 128], FP32, tag="wa")
                for j in range(KD):
                    nc.sync.dma_start(out=wa[:, j * 128:(j + 1) * 128], in_=sfc1b[st, j])
                pa = ps.tile([128, TS], FP32, tag="ps")
                for j in range(KD):
                    nc.tensor.matmul(
                        pa, wa[:, j * 128:(j + 1) * 128],
                        xs_t[:, j * TS:(j + 1) * TS],
                        start=(j == 0), stop=(j == KD - 1),
                    )
                sa = sap.tile([128, TS], FP32, tag="sa")
                nc.scalar.activation(sa, pa, AF.Silu)
                wb = wbp.tile([128, KD * 128], FP32, tag="wb")
                for j in range(KD):
                    nc.sync.dma_start(out=wb[:, j * 128:(j + 1) * 128], in_=sfc2b[st, j])
                pb = ps.tile([128, TS], FP32, tag="ps")
                for j in range(KD):
                    nc.tensor.matmul(
                        pb, wb[:, j * 128:(j + 1) * 128],
                        xs_t[:, j * TS:(j + 1) * TS],
                        start=(j == 0), stop=(j == KD - 1),
                    )
                nc.vector.tensor_mul(g_t[:, st * TS:(st + 1) * TS], sa, pb)

            # owned experts: y_e = silu(x_e @ w1.T) @ w2.T on gathered tokens
            for e in range(EPC):
                xe_t = xep.tile([128, KD * C], FP32, tag="xe")
                for j in range(KD):
                    nc.sync.dma_start(out=xe_t[:, j * C:(j + 1) * C], in_=xg[e, j])
                h_t = hp.tile([128, KH * C], FP32, tag="h")
                for ht in range(KH):
                    w1t = w1p.tile([128, KD * 128], FP32, tag="w1")
                    for j in range(KD):
                        nc.sync.dma_start(
                            out=w1t[:, j * 128:(j + 1) * 128], in_=w1b[e, ht, j]
                        )
                    for off, w in _chunks(C):
                        ph = ps.tile([128, 512], FP32, tag="ps")
                        for j in range(KD):
                            nc.tensor.matmul(
                                ph[:, :w], w1t[:, j * 128:(j + 1) * 128],
                                xe_t[:, j * C + off:j * C + off + w],
                                start=(j == 0), stop=(j == KD - 1),
                            )
                        nc.scalar.activation(
                            h_t[:, ht * C + off:ht * C + off + w], ph[:, :w], AF.Silu
                        )
                for dt in range(KD):
                    w2t = w2p.tile([128, KH * 128], FP32, tag="w2")
                    for j in range(KH):
                        nc.sync.dma_start(
                            out=w2t[:, j * 128:(j + 1) * 128], in_=w2b[e, dt, j]
                        )
                    for off, w in _chunks(C):
                        py = ps.tile([128, 512], FP32, tag="ps")
                        for j in range(KH):
                            nc.tensor.matmul(
                                py[:, :w], w2t[:, j * 128:(j + 1) * 128],
                                h_t[:, j * C + off:j * C + off + w],
                                start=(j == 0), stop=(j == KH - 1),
                            )
                        yo = obp.tile([128, 512], FP32, tag="ob")
                        nc.vector.tensor_copy(yo[:, :w], py[:, :w])
                        nc.sync.dma_start(out=yout[e, dt, :, off:off + w], in_=yo[:, :w])

            # shared second matmul: shared.T[d, t] = sfc3[d, :] @ g[:, t]
            for dt in range(KD):
                w3t = w3p.tile([128, KS * 128], FP32, tag="w3")
                for sj in range(KS):
                    nc.sync.dma_start(
                        out=w3t[:, sj * 128:(sj + 1) * 128], in_=sfc3b[dt, sj]
                    )
                pc = ps.tile([128, TS], FP32, tag="ps")
                for sj in range(KS):
                    nc.tensor.matmul(
                        pc, w3t[:, sj * 128:(sj + 1) * 128],
                        g_t[:, sj * TS:(sj + 1) * TS],
                        start=(sj == 0), stop=(sj == KS - 1),
                    )
                ot = obp.tile([128, TS], FP32, tag="ob")
                nc.vector.tensor_copy(ot, pc)
                nc.sync.dma_start(out=shout[dt], in_=ot)

    nc.compile()
    return nc


def kernel(**inputs):
    global LAST
    x = np.ascontiguousarray(np.asarray(inputs["x"], dtype=np.float32))
    gate_w = np.asarray(inputs["gate_w"], dtype=np.float32)
    w1 = np.asarray(inputs["w1"], dtype=np.float32)
    w2 = np.asarray(inputs["w2"], dtype=np.float32)
    sfc1 = np.asarray(inputs["sfc1"], dtype=np.float32)
    sfc2 = np.asarray(inputs["sfc2"], dtype=np.float32)
    sfc3 = np.asarray(inputs["sfc3"], dtype=np.float32)

    xf = x.reshape(T, D)

    # router on host (tiny): top-2 of 16 logits, softmax over the pair
    logits = xf @ gate_w.T
    idx = np.argpartition(-logits, TOP_K, axis=1)[:, :TOP_K]
    lg = np.take_along_axis(logits, idx, axis=1)
    m = lg.max(axis=1, keepdims=True)
    p = np.exp(lg - m)
    wk = (p / p.sum(axis=1, keepdims=True)).astype(np.float32)

    toks, wts = [], []
    for e in range(E):
        sel = idx == e
        rows = np.nonzero(sel.any(axis=1))[0]
        toks.append(rows)
        wts.append(wk[sel])
    max_load = max(len(r) for r in toks)
    C = max(((max_load + 127) // 128) * 128, 128)

    if C not in _PROG_CACHE:
        _PROG_CACHE[C] = build_program(C)
    nc = _PROG_CACHE[C]

    # shared (replicated) weight blocks
    sfc1b = _block(sfc1.T, 128, 128)           # [KS, KD, 128, 128]
    sfc2b = _block(sfc2.T, 128, 128)
    sfc3b = _block(np.ascontiguousarray(sfc3.T), 128, 128)  # [KD, KS, 128, 128]

    in_maps = []
    for c in range(NCORES):
        xsT = np.ascontiguousarray(xf[c * TS:(c + 1) * TS].T)  # [D, TS]
        xg_c, w1_c, w2_c = [], [], []
        for k in range(EPC):
            e = EPC * c + k
            rows = toks[e]
            xe = np.zeros((C, D), np.float32)
            xe[: len(rows)] = xf[rows]
            xg_c.append(np.ascontiguousarray(xe.T).reshape(KD, 128, C))
            w1_c.append(_block(np.ascontiguousarray(w1[e].T), 128, 128))
            w2_c.append(_block(np.ascontiguousarray(w2[e].T), 128, 128))
        in_maps.append(
            {
                "xs": xsT.reshape(KD, 128, TS),
                "sfc1b": sfc1b,
                "sfc2b": sfc2b,
                "sfc3b": sfc3b,
                "xg": np.stack(xg_c),
                "w1b": np.stack(w1_c),
                "w2b": np.stack(w2_c),
            }
        )

    trace = TRACE or os.environ.get("BASS_TRACE") == "1"
    res = bass_utils.run_bass_kernel_spmd(
        nc, in_maps, core_ids=list(range(NCORES)), trace=trace
    )
    LAST = res
    results = res.results

    out = np.empty((T, D), np.float32)
    for c in range(NCORES):
        shT = np.asarray(results[c]["shout"]).reshape(D, TS)
        out[c * TS:(c + 1) * TS] = shT.T
    for e in range(E):
        c, k = divmod(e, EPC)
        load = len(toks[e])
        yT = np.asarray(results[c]["yout"])[k].reshape(D, C)
        out[toks[e]] += wts[e][:, None] * yT[:, :load].T
    return out.reshape(B, L, D)


# revision 4
# speedup vs baseline: 2.6998x; 2.6998x over previous
"""MoE layer (16 experts, top-2) + shared SwiGLU MLP on 8 trn2 NeuronCores.

Sharding:
  - MoE experts: expert-parallel — core c owns experts {2c, 2c+1}. The host
    computes the router (0.2% of the FLOPs), gathers each expert's tokens
    (the "all-to-all" happens while building per-core inputs), and the device
    runs both expert FFNs on the gathered tokens.
  - Shared SwiGLU MLP: hybrid 4-way token x 2-way hidden shard. Core c
    handles token quarter (c % 4) and S-half (c // 4); each core emits a
    partial second-matmul output and the host sums the two S-halves.
  - The host applies the top-2 softmax combine weights, scatter-adds expert
    outputs, and adds the shared-expert output.

Device matmuls run in float32r (fp32 bits streamed at the PE's 1 cyc/row
rate) with fp32 PSUM accumulation; KMM_DTYPE=fp32|bf16 selects strict fp32
or bf16 operands instead. Every operand is laid out host-side exactly as its
SBUF tile (partition-major), so each DMA is one 0.5-4.2 MB contiguous-row
transfer and every matmul is lhsT.T @ rhs with no on-device transposes.
"""

import os
import numpy as np

import concourse.bacc as bacc
import concourse.mybir as mybir
import concourse.tile as tile
from concourse import bass_utils

AF = mybir.ActivationFunctionType
FP32 = mybir.dt.float32

B, L, D, H, E, S = 2, 2048, 1024, 512, 16, 2048
T = B * L
TOP_K = 2
NCORES = 8
EPC = E // NCORES   # experts per core
PT = 4              # token-shard ways for the shared expert
PS = 2              # hidden(S)-shard ways for the shared expert
TQ = T // PT        # tokens per core for the shared expert (1024)
SH = S // PS        # hidden units per core for the shared expert (1024)

KD = D // 128       # 8 contraction tiles over D
KH = H // 128       # 4 contraction tiles over H
KSH = SH // 128     # 8 s-tiles per core (its S-half)

MM_DTYPE = os.environ.get("KMM_DTYPE", "fp32r")
_MM_DT = {
    "fp32": mybir.dt.float32,
    "fp32r": mybir.dt.float32r,
    "bf16": mybir.dt.bfloat16,
}

TRACE = False      # set True (or BASS_TRACE=1) to collect an NTFF profile
LAST = None        # BassKernelResults of the most recent run (for test.py)

_PROG_CACHE = {}


def _chunks(total, step=512):
    """Split ``total`` into near-equal chunks <= step (keeps chunks >= 256
    when possible so float32r matmuls stay at the 1 cyc/row rate)."""
    n = max(1, -(-total // step))
    base = total // n
    rem = total - base * n
    out, off = [], 0
    for i in range(n):
        w = base + (1 if i < rem else 0)
        out.append((off, w))
        off += w
    return out


def _pmajor(a, cols):
    """[K, M] k-major matrix -> [128, (K/128)*M] partition-major image whose
    columns are the K-tiles side by side; ``cols`` = M per tile."""
    K, M = a.shape
    assert M == cols
    return np.ascontiguousarray(
        a.reshape(K // 128, 128, M).transpose(1, 0, 2).reshape(128, -1)
    )


def build_program(C, mmdt_key=None):
    mmdt = _MM_DT[mmdt_key or MM_DTYPE]
    nc = bacc.Bacc(
        "TRN2", target_bir_lowering=False, debug=False, enable_asserts=False
    )

    xq = nc.dram_tensor("xq", [128, KD * TQ], mmdt, kind="ExternalInput").ap()
    # per s-tile: 8 sfc1 k-tiles then 8 sfc2 k-tiles, side by side
    sfc12 = nc.dram_tensor("sfc12", [KSH, 128, 2 * KD * 128], mmdt, kind="ExternalInput").ap()
    # per d-tile: the core's 8 sfc3 s-tiles
    sfc3h = nc.dram_tensor("sfc3h", [KD, 128, KSH * 128], mmdt, kind="ExternalInput").ap()
    xg = nc.dram_tensor("xg", [EPC, 128, KD * C], mmdt, kind="ExternalInput").ap()
    w1b = nc.dram_tensor("w1b", [EPC, 128, KH * KD * 128], mmdt, kind="ExternalInput").ap()
    w2b = nc.dram_tensor("w2b", [EPC, 128, KD * KH * 128], mmdt, kind="ExternalInput").ap()
    pshout = nc.dram_tensor("pshout", [KD, 128, TQ], FP32, kind="ExternalOutput").ap()
    yout = nc.dram_tensor("yout", [EPC, KD, 128, C], FP32, kind="ExternalOutput").ap()

    tch = _chunks(TQ)   # token chunks for the shared expert (2 x 512)
    cch = _chunks(C)    # token chunks for the owned experts

    with tile.TileContext(nc) as tc:
        with (
            tc.tile_pool(name="xqp", bufs=1) as xqp,
            tc.tile_pool(name="gp", bufs=1) as gp,
            tc.tile_pool(name="w12p", bufs=2) as w12p,
            tc.tile_pool(name="w3p", bufs=2) as w3p,
            tc.tile_pool(name="w1p", bufs=2) as w1p,
            tc.tile_pool(name="w2p", bufs=2) as w2p,
            tc.tile_pool(name="xep", bufs=1) as xep,
            tc.tile_pool(name="hp", bufs=1) as hp,
            tc.tile_pool(name="sap", bufs=3) as sap,
            tc.tile_pool(name="obp", bufs=4) as obp,
            tc.tile_pool(name="ps", bufs=6, space="PSUM") as ps,
        ):
            # resident token quarter, d-major
            xq_t = xqp.tile([128, KD * TQ], mmdt)
            nc.sync.dma_start(out=xq_t[:], in_=xq[:])

            # g[s, t] = silu(x@sfc1.T) * (x@sfc2.T) for this core's S-half
            g_t = gp.tile([128, KSH * TQ], mmdt)
            for st in range(KSH):
                w12 = w12p.tile([128, 2 * KD * 128], mmdt, tag="w12")
                nc.sync.dma_start(out=w12[:], in_=sfc12[st])
                for off, w in tch:
                    pa = ps.tile([128, 512], FP32, tag="ps")
                    for j in range(KD):
                        nc.tensor.matmul(
                            pa[:, :w], w12[:, j * 128:(j + 1) * 128],
                            xq_t[:, j * TQ + off:j * TQ + off + w],
                            start=(j == 0), stop=(j == KD - 1),
                        )
                    sa = sap.tile([128, 512], FP32, tag="sa")
                    nc.scalar.activation(sa[:, :w], pa[:, :w], AF.Silu)
                    pb = ps.tile([128, 512], FP32, tag="ps")
                    for j in range(KD):
                        nc.tensor.matmul(
                            pb[:, :w], w12[:, (KD + j) * 128:(KD + j + 1) * 128],
                            xq_t[:, j * TQ + off:j * TQ + off + w],
                            start=(j == 0), stop=(j == KD - 1),
                        )
                    nc.vector.tensor_mul(
                        g_t[:, st * TQ + off:st * TQ + off + w], sa[:, :w], pb[:, :w]
                    )

            # owned experts: y_e = silu(x_e @ w1.T) @ w2.T on gathered tokens
            for e in range(EPC):
                xe_t = xep.tile([128, KD * C], mmdt, tag="xe")
                nc.sync.dma_start(out=xe_t[:], in_=xg[e])
                h_t = hp.tile([128, KH * C], mmdt, tag="h")
                half = KH * KD * 128 // 2
                w1t = [None, None]
                for hf in range(2):
                    w1t[hf] = w1p.tile([128, half], mmdt, tag="w1", name=f"w1t{e}_{hf}")
                    nc.sync.dma_start(
                        out=w1t[hf][:], in_=w1b[e, :, hf * half:(hf + 1) * half]
                    )
                for ht in range(KH):
                    hf, hb = divmod(ht, KH // 2)
                    for off, w in cch:
                        ph = ps.tile([128, 512], FP32, tag="ps")
                        for j in range(KD):
                            nc.tensor.matmul(
                                ph[:, :w],
                                w1t[hf][:, (hb * KD + j) * 128:(hb * KD + j + 1) * 128],
                                xe_t[:, j * C + off:j * C + off + w],
                                start=(j == 0), stop=(j == KD - 1),
                            )
                        nc.scalar.activation(
                            h_t[:, ht * C + off:ht * C + off + w], ph[:, :w], AF.Silu
                        )
                w2t = [None, None]
                for hf in range(2):
                    w2t[hf] = w2p.tile([128, half], mmdt, tag="w2", name=f"w2t{e}_{hf}")
                    nc.sync.dma_start(
                        out=w2t[hf][:], in_=w2b[e, :, hf * half:(hf + 1) * half]
                    )
                for dt in range(KD):
                    hf, db = divmod(dt, KD // 2)
                    for off, w in cch:
                        py = ps.tile([128, 512], FP32, tag="ps")
                        for j in range(KH):
                            nc.tensor.matmul(
                                py[:, :w],
                                w2t[hf][:, (db * KH + j) * 128:(db * KH + j + 1) * 128],
                                h_t[:, j * C + off:j * C + off + w],
                                start=(j == 0), stop=(j == KH - 1),
                            )
                        yo = obp.tile([128, 512], FP32, tag="ob")
                        nc.vector.tensor_copy(yo[:, :w], py[:, :w])
                        nc.scalar.dma_start(
                            out=yout[e, dt, :, off:off + w], in_=yo[:, :w]
                        )

            # partial shared second matmul over this core's S-half:
            # pshout[d, t] = sum_{s in half} sfc3[d, s] * g[s, t]
            for dt in range(KD):
                w3t = w3p.tile([128, KSH * 128], mmdt, tag="w3")
                nc.sync.dma_start(out=w3t[:], in_=sfc3h[dt])
                po = obp.tile([128, TQ], FP32, tag="po")
                for off, w in tch:
                    pc = ps.tile([128, 512], FP32, tag="ps")
                    for sj in range(KSH):
                        nc.tensor.matmul(
                            pc[:, :w], w3t[:, sj * 128:(sj + 1) * 128],
                            g_t[:, sj * TQ + off:sj * TQ + off + w],
                            start=(sj == 0), stop=(sj == KSH - 1),
                        )
                    nc.vector.tensor_copy(po[:, off:off + w], pc[:, :w])
                nc.scalar.dma_start(out=pshout[dt], in_=po[:])

    nc.compile()
    return nc


def kernel(**inputs):
    global LAST
    x = np.ascontiguousarray(np.asarray(inputs["x"], dtype=np.float32))
    gate_w = np.asarray(inputs["gate_w"], dtype=np.float32)
    w1 = np.asarray(inputs["w1"], dtype=np.float32)
    w2 = np.asarray(inputs["w2"], dtype=np.float32)
    sfc1 = np.asarray(inputs["sfc1"], dtype=np.float32)
    sfc2 = np.asarray(inputs["sfc2"], dtype=np.float32)
    sfc3 = np.asarray(inputs["sfc3"], dtype=np.float32)

    xf = x.reshape(T, D)

    # router on host (tiny): top-2 of 16 logits, softmax over the pair
    logits = xf @ gate_w.T
    idx = np.argpartition(-logits, TOP_K, axis=1)[:, :TOP_K]
    lg = np.take_along_axis(logits, idx, axis=1)
    m = lg.max(axis=1, keepdims=True)
    p = np.exp(lg - m)
    wk = (p / p.sum(axis=1, keepdims=True)).astype(np.float32)

    toks, wts = [], []
    for e in range(E):
        sel = idx == e
        rows = np.nonzero(sel.any(axis=1))[0]
        toks.append(rows)
        wts.append(wk[sel])
    max_load = max(len(r) for r in toks)
    C = max(((max_load + 127) // 128) * 128, 256)

    key = (C, MM_DTYPE)
    if key not in _PROG_CACHE:
        _PROG_CACHE[key] = build_program(C, MM_DTYPE)
    nc = _PROG_CACHE[key]
    np_mm = mybir.dt.np(_MM_DT[MM_DTYPE])

    sfc1T = np.ascontiguousarray(sfc1.T)   # [D, S]
    sfc2T = np.ascontiguousarray(sfc2.T)
    sfc3T = np.ascontiguousarray(sfc3.T)   # [S, D]

    # sfc12 per S-half: [KSH, 128, 2*KD*128]
    sfc12_h, sfc3_h = [], []
    for sh in range(PS):
        blk = np.empty((KSH, 128, 2 * KD * 128), np.float32)
        for st in range(KSH):
            s0 = (sh * KSH + st) * 128
            a = sfc1T[:, s0:s0 + 128]    # [D, 128]
            b = sfc2T[:, s0:s0 + 128]
            blk[st, :, : KD * 128] = _pmajor(a, 128)
            blk[st, :, KD * 128:] = _pmajor(b, 128)
        sfc12_h.append(blk.astype(np_mm))
        blk3 = np.empty((KD, 128, KSH * 128), np.float32)
        s0 = sh * SH
        for dt in range(KD):
            # [SH, 128] slice of sfc3T -> partition-major over its s-tiles
            blk3[dt] = _pmajor(
                np.ascontiguousarray(sfc3T[s0:s0 + SH, dt * 128:(dt + 1) * 128]), 128
            )
        sfc3_h.append(blk3.astype(np_mm))

    in_maps = []
    for c in range(NCORES):
        q, sh = c % PT, c // PT
        xqm = _pmajor(
            np.ascontiguousarray(xf[q * TQ:(q + 1) * TQ].T), TQ
        ).astype(np_mm)
        xg_c, w1_c, w2_c = [], [], []
        for k in range(EPC):
            e = EPC * c + k
            rows = toks[e]
            xe = np.zeros((C, D), np.float32)
            xe[: len(rows)] = xf[rows]
            xg_c.append(_pmajor(np.ascontiguousarray(xe.T), C))
            # w1 tiles keyed (ht, j): col block (ht*KD + j) is k-tile j of
            # w1[e].T's h-tile ht
            w1T = np.ascontiguousarray(w1[e].T)   # [D, H]
            w1m = np.empty((128, KH * KD * 128), np.float32)
            for ht in range(KH):
                w1m[:, ht * KD * 128:(ht + 1) * KD * 128] = _pmajor(
                    np.ascontiguousarray(w1T[:, ht * 128:(ht + 1) * 128]), 128
                )
            w1_c.append(w1m)
            # w2 tiles keyed (dt, hj)
            w2T = np.ascontiguousarray(w2[e].T)   # [H, D]
            w2m = np.empty((128, KD * KH * 128), np.float32)
            for dt in range(KD):
                w2m[:, dt * KH * 128:(dt + 1) * KH * 128] = _pmajor(
                    np.ascontiguousarray(w2T[:, dt * 128:(dt + 1) * 128]), 128
                )
            w2_c.append(w2m)
        in_maps.append(
            {
                "xq": xqm,
                "sfc12": sfc12_h[sh],
                "sfc3h": sfc3_h[sh],
                "xg": np.stack(xg_c).astype(np_mm),
                "w1b": np.stack(w1_c).astype(np_mm),
                "w2b": np.stack(w2_c).astype(np_mm),
            }
        )

    trace = TRACE or os.environ.get("BASS_TRACE") == "1"
    res = bass_utils.run_bass_kernel_spmd(
        nc, in_maps, core_ids=list(range(NCORES)), trace=trace
    )
    LAST = res
    results = res.results

    out = np.empty((T, D), np.float32)
    for q in range(PT):
        acc = np.asarray(results[q]["pshout"], np.float32).reshape(D, TQ)
        acc = acc + np.asarray(results[PT + q]["pshout"], np.float32).reshape(D, TQ)
        out[q * TQ:(q + 1) * TQ] = acc.T
    for e in range(E):
        c, k = divmod(e, EPC)
        load = len(toks[e])
        yT = np.asarray(results[c]["yout"])[k].reshape(D, C)
        out[toks[e]] += wts[e][:, None] * yT[:, :load].T
    return out.reshape(B, L, D)
